# revision 1
# baseline (speedup 1.0000x reference)
"""Trainium2 Bass kernel for nn_DenseModel_51926154609008 (weighted-rank
contrastive CE loss).

Math (reference semantics, no sort needed):
  scores = q @ p.T                       [B=2048, P=16384]
  t_i    = scores[i, 8*i]                (positive/target score)
  rank_i = #{j : scores[i, j] > t_i}     (argsort position == exceed count,
                                          ties are measure-zero for randn data)
  lse_i  = logsumexp(scores[i, :])
  loss   = mean((lse_i - t_i) * (1 + 2.6*exp(-(rank_i-1)^2 / (2*1.8^2))))

Sharding: passage-parallel (P split across 8 cores, q replicated) — 12.6MB
of HBM reads per core vs 51MB for query-parallel with replicated passages.
Each core computes a [2048, 2048] score slab in 32 half-tiles
([128 queries x 1024 passages], one 2-bank PSUM buffer each) and reduces
every half-tile to per-query partials:
  sumexp_c[i] = sum_j exp(s_ij - C)      (fixed shift C so partials add
                                          across cores without a max-merge)
  cnt_c[i]    = #{j in slab : s_ij > t_i}
The host combines partials and evaluates the tiny [2048] tail in fp64.

The self-comparison (j == 8i) must contribute exactly 0 to rank_i. Query i's
target column lives only in core (i//256)'s slab. Each core rotates its query
order (data-level permutation — the program stays SPMD-uniform) so its own
queries always land on m-tiles OWN_M, OWN_M+1; the two half-tiles containing
self-columns use a masked count (indicator * mask, one fused DVE op); all
other half-tiles use a plain per-partition is_gt count.

t itself is computed on the host (trivial 2048x768 row-dot).

HW notes baked in from trace/bisect evidence:
  - DVE ops fault when an access pattern spans >2 PSUM banks; 2 banks is
    fine -> [128, 1024] half-tiles, one count op each.
  - ACT reads spanning 4 banks are fine; exp uses the per-instruction
    accumulator (sum along free dim) so no junk reduction is needed.
  - Mixing ACT functions (Exp/Sigmoid) forces ~1.3us ACT_TABLE_LOADs; the
    kernel uses Exp only.
  - bf16 matmuls stream at ~216ns per [128x512] MM warm; fp32 runs 2x
    slower and float32r ~1.9x (fp32_mode=HIGH, no FWL weight loads).
  - Input DMAs are split into [128, 512] sub-chunks, ordered so the first
    m-tile's operands land first (whole-tile DMAs starved the PE for ~14us).
"""

import sys

import numpy as np

sys.path.insert(0, "/opt/trn_rl_repo")

import concourse.bacc as bacc  # noqa: E402
import concourse.bass as bass  # noqa: E402
import concourse.mybir as mybir  # noqa: E402
import concourse.tile as tile  # noqa: E402
from concourse.bass_utils import run_bass_kernel_spmd  # noqa: E402

# Problem shape (hardcoded per the task contract).
B = 2048
D = 768
NP = 8
P = B * NP  # 16384
NCORES = 8
PSLAB = P // NCORES  # 2048 passage columns per core
KCH = D // 128  # 6 contraction chunks
MT = B // 128  # 16 query m-tiles
NU = 2 * MT  # 32 half-tile units of [128, 1024]
QSLAB = B // NCORES  # 256 queries owned per core
OWN_M = 8  # own queries sit at m-tiles 8,9 (mask off the critical path)

C_SHIFT = 128.0  # fixed exp shift: exp(s - C) never overflows for this data

ALPHA = 2.6
OPTIMAL_RANK = 1.0
SIGMA = 1.8

# Matmul input dtype: bfloat16 | float32r | float32
MM_DT = mybir.dt.bfloat16

_STATE: dict = {}


def _build_nc(mm_dt):
    nc = bacc.Bacc("TRN2", target_bir_lowering=False, debug=False,
                   num_devices=NCORES)

    qT_d = nc.dram_tensor("qT", [D, B], mm_dt, kind="ExternalInput").ap()
    pT_d = nc.dram_tensor("pT", [D, PSLAB], mm_dt, kind="ExternalInput").ap()
    tv_d = nc.dram_tensor("tvec", [128, MT], mybir.dt.float32,
                          kind="ExternalInput").ap()
    msk_d = nc.dram_tensor("msk", [128, 1024], mybir.dt.float32,
                           kind="ExternalInput").ap()
    se_d = nc.dram_tensor("se_out", [128, 2 * NU], mybir.dt.float32,
                          kind="ExternalOutput").ap()
    cnt_d = nc.dram_tensor("cnt_out", [128, 2 * NU], mybir.dt.float32,
                           kind="ExternalOutput").ap()

    f32 = mybir.dt.float32
    bf16 = mybir.dt.bfloat16

    with tile.TileContext(nc) as tc:
        with (
            tc.tile_pool(name="weights", bufs=1) as wpool,
            tc.tile_pool(name="stats", bufs=1) as spool,
            tc.tile_pool(name="junk", bufs=3) as jpool,
            tc.tile_pool(name="psum", bufs=4,
                         space=bass.MemorySpace.PSUM) as ppool,
        ):
            qk = [wpool.tile([128, B], mm_dt, name=f"qk{k}", tag=f"qk{k}")
                  for k in range(KCH)]
            pk = [wpool.tile([128, PSLAB], mm_dt, name=f"pk{k}", tag=f"pk{k}")
                  for k in range(KCH)]

            def ldq(k, part):  # issued on GpSimd's sequencer
                nc.gpsimd.dma_start(
                    qk[k][:, part * 512:(part + 1) * 512],
                    qT_d[k * 128:(k + 1) * 128, part * 512:(part + 1) * 512])

            def ldp(k, half):  # issued on Sync's sequencer
                nc.sync.dma_start(
                    pk[k][:, half * 1024:(half + 1) * 1024],
                    pT_d[k * 128:(k + 1) * 128, half * 1024:(half + 1) * 1024])

            # Units run nh-major (all half-0 m-tiles, then all half-1), so
            # pk half 1 isn't needed until mid-kernel; qk part p feeds
            # m-tiles 4p..4p+3. DMA issue is ~0.6us per dma_start on the
            # issuing sequencer, so the critical first operands go first,
            # split across two sequencers (Sync: pk, GpSimd: qk).
            tv = spool.tile([128, MT], f32, name="tv", tag="tv")
            msk = spool.tile([128, 1024], f32, name="msk", tag="msk")
            # smallest-possible first dependencies: MM#0 needs qk0 cols
            # 0:128 (LDWEIGHTS) and pk0 cols 0:512 only
            nc.gpsimd.dma_start(qk[0][:, 0:128], qT_d[0:128, 0:128])
            nc.sync.dma_start(pk[0][:, 0:512], pT_d[0:128, 0:512])
            nc.gpsimd.dma_start(qk[0][:, 128:512], qT_d[0:128, 128:512])
            nc.sync.dma_start(pk[0][:, 512:1024], pT_d[0:128, 512:1024])
            for k in range(1, KCH):
                ldq(k, 0)
                if k <= 3:
                    nc.sync.dma_start(pk[k][:, 0:1024],
                                      pT_d[k * 128:(k + 1) * 128, 0:1024])
                else:
                    # third sequencer so the k-chain of first-unit operands
                    # arrives faster than the PE consumes it
                    nc.scalar.dma_start(pk[k][:, 0:1024],
                                        pT_d[k * 128:(k + 1) * 128, 0:1024])
            nc.sync.dma_start(tv[:], tv_d[:])
            for k in range(KCH):
                ldq(k, 1)
            nc.gpsimd.dma_start(msk[:], msk_d[:])
            for k in range(KCH):
                ldp(k, 1)
                ldq(k, 2)
            for k in range(KCH):
                ldq(k, 3)

            se_sb = spool.tile([128, 2 * NU], f32, name="se_sb", tag="se_sb")
            cnt_sb = spool.tile([128, 2 * NU], f32, name="cnt_sb",
                                tag="cnt_sb")
            negc = spool.tile([128, 1], f32, name="negc", tag="negc")
            nc.vector.memset(negc[:], -C_SHIFT)

            for u in range(NU):
                nh, m = u // MT, u % MT
                ps = ppool.tile([128, 1024], f32, name="ps", tag="ps")
                for nloc in range(2):
                    nb = nh * 2 + nloc
                    for k in range(KCH):
                        nc.tensor.matmul(
                            ps[:, nloc * 512:(nloc + 1) * 512],
                            qk[k][:, m * 128:(m + 1) * 128],
                            pk[k][:, nb * 512:(nb + 1) * 512],
                            start=(k == 0),
                            stop=(k == KCH - 1),
                        )
                # per-bank stats: each 512-col bank's reducers fire as soon
                # as its 6-MM accumulation finishes, halving the PSUM
                # slot-release lag behind the PE.
                je = jpool.tile([128, 1024], bf16, name="je", tag="je")
                jc = jpool.tile([128, 1024], bf16, name="jc", tag="jc")
                for h in range(2):
                    sl = slice(h * 512, (h + 1) * 512)
                    col = 2 * u + h
                    nc.scalar.activation(
                        je[:, sl], ps[:, sl],
                        mybir.ActivationFunctionType.Exp,
                        bias=negc[:], scale=1.0,
                        accum_out=se_sb[:, col:col + 1],
                    )
                    if u in (OWN_M, MT + OWN_M + 1):
                        # half-tiles holding the self column: masked count
                        nc.vector.scalar_tensor_tensor(
                            out=jc[:, sl], in0=ps[:, sl],
                            scalar=tv[:, m:m + 1], in1=msk[:, sl],
                            op0=mybir.AluOpType.is_gt,
                            op1=mybir.AluOpType.mult,
                            accum_out=cnt_sb[:, col:col + 1],
                        )
                    else:
                        nc.vector.tensor_scalar(
                            jc[:, sl], ps[:, sl], tv[:, m:m + 1], None,
                            op0=mybir.AluOpType.is_gt,
                            op1=mybir.AluOpType.add,
                            accum_out=cnt_sb[:, col:col + 1],
                        )

            nc.sync.dma_start(se_d[:], se_sb[:])
            nc.gpsimd.dma_start(cnt_d[:], cnt_sb[:])

    nc.compile()
    return nc


def _np_dtype(mm_dt):
    if mm_dt == mybir.dt.bfloat16:
        import ml_dtypes
        return ml_dtypes.bfloat16
    return np.float32


def _perm(c):
    """Rotation putting core c's own queries at m-tiles OWN_M, OWN_M+1."""
    return np.roll(np.arange(B), OWN_M * 128 - c * QSLAB)


def prepare(q, p, mm_dt=None):
    """Host-side shard prep. Returns (in_maps, t32, perms)."""
    if mm_dt is None:
        mm_dt = MM_DT
    npdt = _np_dtype(mm_dt)
    q = np.ascontiguousarray(np.asarray(q, dtype=np.float32))
    p = np.ascontiguousarray(np.asarray(p, dtype=np.float32))

    # target scores t_i = q_i . p_{8i} (fp32; matches the reference's fp32
    # value to ~1e-7 — only a compare threshold + host-tail term)
    t32 = np.einsum("ij,ij->i", q, p[::NP], dtype=np.float64).astype(np.float32)

    qT = np.ascontiguousarray(q.T)  # [D, B] fp32
    r = np.arange(128)
    # self columns: unit 2*OWN_M has query pi=OWN_M*128+r vs local col 8r
    # (half 0); unit 2*OWN_M+3 has pi=(OWN_M+1)*128+r vs col 1024+8r
    # (i.e. col 8r of half 1). Same mask for both, same for every core.
    msk = np.ones((128, 1024), dtype=np.float32)
    msk[r, 8 * r] = 0.0

    in_maps = []
    perms = []
    for c in range(NCORES):
        perm = _perm(c)
        perms.append(perm)
        qTc = np.ascontiguousarray(qT[:, perm]).astype(npdt)
        pTc = np.ascontiguousarray(p[c * PSLAB:(c + 1) * PSLAB].T).astype(npdt)
        tvc = np.ascontiguousarray(t32[perm].reshape(MT, 128).T)
        in_maps.append({"qT": qTc, "pT": pTc, "tvec": tvc, "msk": msk})
    return in_maps, t32, perms


def finalize(results, t32, perms):
    """Combine per-core partials into the scalar loss (fp64 host tail)."""
    se_tot = np.zeros(B, dtype=np.float64)
    cnt_tot = np.zeros(B, dtype=np.float64)
    for c in range(NCORES):
        perm = perms[c]
        # column c = 32*nh + 2*m + h; query pi = m*128 + r
        se = results[c]["se_out"].astype(np.float64)
        cnt = results[c]["cnt_out"].astype(np.float64)
        se_q = se.reshape(128, 2, MT, 2).sum(axis=(1, 3)).T.ravel()
        cnt_q = cnt.reshape(128, 2, MT, 2).sum(axis=(1, 3)).T.ravel()
        se_tot[perm] += se_q
        cnt_tot[perm] += cnt_q
    lse = C_SHIFT + np.log(se_tot)
    raw = lse - t32.astype(np.float64)
    w = 1.0 + ALPHA * np.exp(-((cnt_tot - OPTIMAL_RANK) ** 2)
                             / (2.0 * SIGMA ** 2))
    return np.float32(np.mean(raw * w))


def _get_nc(mm_dt=None):
    if mm_dt is None:
        mm_dt = MM_DT
    if mm_dt not in _STATE:
        _STATE[mm_dt] = _build_nc(mm_dt)
    return _STATE[mm_dt]


def kernel(q_reps, p_reps, n_passages):
    assert int(np.asarray(n_passages)) == NP
    nc = _get_nc()
    in_maps, t32, perms = prepare(q_reps, p_reps)
    try:
        res = run_bass_kernel_spmd(nc, in_maps, core_ids=list(range(NCORES)))
    except Exception:
        # rare transient NRT_EXEC_UNIT_UNRECOVERABLE; reset the PJRT
        # client and retry once
        import time
        try:
            import jax
            jax.clear_caches()
            jax.extend.backend.clear_backends()
        except Exception:
            pass
        time.sleep(10)
        res = run_bass_kernel_spmd(nc, in_maps, core_ids=list(range(NCORES)))
    return finalize(res.results, t32, perms)


def run_profiled(q_reps, p_reps, n_passages, mm_dt=None, trace=True):
    """Same as kernel() but returns (loss, BassKernelResults) with NTFF
    profile (requires the antenv.axon_hooks shim; see _install_ntff_shim)."""
    nc = _get_nc(mm_dt)
    in_maps, t32, perms = prepare(q_reps, p_reps, mm_dt)
    res = run_bass_kernel_spmd(nc, in_maps, core_ids=list(range(NCORES)),
                               trace=trace)
    loss = finalize(res.results, t32, perms)
    return loss, res


def _install_ntff_shim():
    """Provide antenv.axon_hooks (absent in this image) so trace=True works."""
    import types
    import antenv
    if "antenv.axon_hooks" in sys.modules:
        return
    mod = types.ModuleType("antenv.axon_hooks")
    mod._hook = None
    mod.set_axon_ntff_profile_hook = lambda h: setattr(mod, "_hook", h)
    mod.get_axon_ntff_profile_hook = lambda: mod._hook
    sys.modules["antenv.axon_hooks"] = mod
    antenv.axon_hooks = mod
    try:
        from trn_agent_boot.trn_boot import _ntff_profile_via_ctypes
        hook = _ntff_profile_via_ctypes("/opt/axon/libaxon_pjrt.so")
        if hook is not None:
            mod._hook = hook
    except Exception:
        pass



# revision 6
# speedup vs baseline: 1.5677x; 1.5677x over previous
"""Trainium2 Bass kernel for nn_DenseModel_51926154609008 (weighted-rank
contrastive CE loss).

Math (reference semantics, no sort needed):
  scores = q @ p.T                       [B=2048, P=16384]
  t_i    = scores[i, 8*i]                (positive/target score)
  rank_i = #{j : scores[i, j] > t_i}     (argsort position == exceed count,
                                          ties are measure-zero for randn data)
  lse_i  = logsumexp(scores[i, :])
  loss   = mean((lse_i - t_i) * (1 + 2.6*exp(-(rank_i-1)^2 / (2*1.8^2))))

Sharding: passage-parallel (P split across 8 cores, q replicated). Each core
computes a [2048, 2048] score slab in 32 units of [128 queries x 1024
passages] (one 2-bank PSUM buffer each) and reduces every unit to per-query
partials:
  sumexp_c[i] = sum_j exp(s_ij - C)      (fixed shift C so partials add
                                          across cores without a max-merge)
  cnt_c[i]    = #{j in slab : s_ij > t_i}
The host combines partials and evaluates the tiny [2048] tail in fp64.

Matmuls run in fp8-e4m3 with MatmulPerfMode.DoubleRow: one MM contracts TWO
128-deep k-chunks (lhsT [128,2,128], rhs [128,2,512] -> out [128,512] PSUM),
2x the bf16 PE throughput. End-to-end fp8 quantization error on this exact
input is rel 3.4e-4 on the loss (verified numerically) vs the 2e-2 gate.

Per unit, stats run at full [128,1024] granularity (2 PSUM banks):
  - ACT: exp(s - C) with sum-accumulator -> se col (one op, ~1.2us)
  - DVE: is_gt vs t with sum-accumulator -> cnt col (one op, ~1.2us).
    (Pool/GPSIMD cannot read PSUM, so it can't share this work.)
Both fit under the fp8 PE unit time (~1.3us) so the kernel stays PE-bound.

The self-comparison (j == 8i) must contribute exactly 0 to rank_i. Query i's
target column lives only in core (i//256)'s slab. Each core rotates its query
order (data-level permutation, program stays SPMD-uniform) so its own queries
land on m-tiles OWN_M, OWN_M+1; the two units containing self-columns use a
masked count (indicator * mask, one fused DVE op).

t itself is computed on the host (trivial 2048x768 row-dot, exact fp32).

HW notes baked in from trace/bisect evidence (previous sessions):
  - DVE ops fault when an access pattern spans >2 PSUM banks; 2 banks is
    fine -> one [128,1024] count op per unit.
  - ACT reads spanning 4 banks are fine; exp uses the per-instruction
    accumulator (sum along free dim) so no junk reduction is needed.
  - Mixing ACT functions forces ~1.3us ACT_TABLE_LOADs; Exp only here.
  - DMA issue is ~0.6us per dma_start on the issuing sequencer, so the
    critical first operands go first, split across four sequencers.
"""

import sys

import numpy as np

sys.path.insert(0, "/opt/trn_rl_repo")

import concourse.bacc as bacc  # noqa: E402
import concourse.bass as bass  # noqa: E402
import concourse.mybir as mybir  # noqa: E402
import concourse.tile as tile  # noqa: E402
from concourse.bass_utils import run_bass_kernel_spmd  # noqa: E402

# Problem shape (hardcoded per the task contract).
B = 2048
D = 768
NP = 8
P = B * NP  # 16384
NCORES = 8
PSLAB = P // NCORES  # 2048 passage columns per core
KP = D // 256  # 3 DoubleRow k-pairs (each = two 128-deep chunks)
MT = B // 128  # 16 query m-tiles
NU = 2 * MT  # 32 units of [128, 1024]
QSLAB = B // NCORES  # 256 queries owned per core
OWN_M = 8  # own queries sit at m-tiles 8,9
MASK_UNITS = (OWN_M, MT + OWN_M + 1)  # units holding self-columns

C_SHIFT = 128.0  # fixed exp shift: exp(s - C) never overflows for this data

ALPHA = 2.6
OPTIMAL_RANK = 1.0
SIGMA = 1.8

_STATE: dict = {}


def _build_nc():
    nc = bacc.Bacc("TRN2", target_bir_lowering=False, debug=False,
                   num_devices=NCORES)

    fp8 = mybir.dt.float8e4
    f32 = mybir.dt.float32
    bf16 = mybir.dt.bfloat16
    DR = mybir.MatmulPerfMode.DoubleRow

    qT_d = nc.dram_tensor("qT", [KP, 2, 128, B], fp8, kind="ExternalInput").ap()
    pT_d = nc.dram_tensor("pT", [KP, 2, 128, PSLAB], fp8,
                          kind="ExternalInput").ap()
    tv_d = nc.dram_tensor("tvec", [128, MT], f32, kind="ExternalInput").ap()
    msk_d = nc.dram_tensor("msk", [128, 1024], bf16, kind="ExternalInput").ap()
    se_d = nc.dram_tensor("se_out", [128, NU], f32, kind="ExternalOutput").ap()
    cnt_d = nc.dram_tensor("cnt_out", [128, NU], f32,
                           kind="ExternalOutput").ap()

    with tile.TileContext(nc) as tc:
        with (
            tc.tile_pool(name="weights", bufs=1) as wpool,
            tc.tile_pool(name="stats", bufs=1) as spool,
            tc.tile_pool(name="junk", bufs=3) as jpool,
            tc.tile_pool(name="psum", bufs=4,
                         space=bass.MemorySpace.PSUM) as ppool,
        ):
            qk = [wpool.tile([128, 2, B], fp8, name=f"qk{k}", tag=f"qk{k}")
                  for k in range(KP)]
            pk = [wpool.tile([128, 2, PSLAB], fp8, name=f"pk{k}", tag=f"pk{k}")
                  for k in range(KP)]
            tv = spool.tile([128, MT], f32, name="tv", tag="tv")
            msk = spool.tile([128, 1024], bf16, name="msk", tag="msk")
            negc = spool.tile([128, 1], f32, name="negc", tag="negc")
            se_sb = spool.tile([128, NU], f32, name="se_sb", tag="se_sb")
            cnt_sb = spool.tile([128, NU], f32, name="cnt_sb", tag="cnt_sb")

            def ldq(eng, kk, i, c0, c1):
                eng.dma_start(qk[kk][:, i, c0:c1], qT_d[kk, i, :, c0:c1])

            def ldp(eng, kk, i, c0, c1):
                eng.dma_start(pk[kk][:, i, c0:c1], pT_d[kk, i, :, c0:c1])

            # negc must precede the first ACT op; DVE engine op, trivial.
            nc.vector.memset(negc[:], -C_SHIFT)

            # DMA issue is ~0.6us per dma_start on its sequencer, and only
            # sync/scalar/gpsimd sequencers can issue. Order per sequencer
            # by first-use time: unit u=(nh,m) consumes pk cols
            # [nh*1024, (nh+1)*1024) (both i) and qk cols [m*128,(m+1)*128);
            # units run nh-major.
            # sync: bank-0 pk (first unit's first 3 MMs) + tv + late pk i=0
            ldp(nc.sync, 0, 0, 0, 512)
            ldp(nc.sync, 0, 1, 0, 512)
            ldp(nc.sync, 1, 0, 0, 512)
            ldp(nc.sync, 1, 1, 0, 512)
            nc.sync.dma_start(tv[:], tv_d[:])
            ldp(nc.sync, 2, 0, 0, 512)
            ldp(nc.sync, 2, 1, 0, 512)
            ldp(nc.sync, 0, 0, 1024, 2048)
            ldp(nc.sync, 1, 0, 1024, 2048)
            ldp(nc.sync, 2, 0, 1024, 2048)
            # scalar: bank-1 pk + msk, then the sequencer is clear for ACT
            ldp(nc.scalar, 0, 0, 512, 1024)
            ldp(nc.scalar, 0, 1, 512, 1024)
            ldp(nc.scalar, 1, 0, 512, 1024)
            ldp(nc.scalar, 1, 1, 512, 1024)
            ldp(nc.scalar, 2, 0, 512, 1024)
            ldp(nc.scalar, 2, 1, 512, 1024)
            nc.scalar.dma_start(msk[:], msk_d[:])
            # gpsimd: all qk chunks (m0-1, then m2-7, then m8-15), late pk
            for kk in range(KP):
                ldq(nc.gpsimd, kk, 0, 0, 256)
                ldq(nc.gpsimd, kk, 1, 0, 256)
            for kk in range(KP):
                ldq(nc.gpsimd, kk, 0, 256, 1024)
                ldq(nc.gpsimd, kk, 1, 256, 1024)
            for kk in range(KP):
                ldq(nc.gpsimd, kk, 0, 1024, 2048)
                ldq(nc.gpsimd, kk, 1, 1024, 2048)
            ldp(nc.gpsimd, 0, 1, 1024, 2048)
            ldp(nc.gpsimd, 1, 1, 1024, 2048)
            ldp(nc.gpsimd, 2, 1, 1024, 2048)

            for u in range(NU):
                nh, m = u // MT, u % MT
                ps = ppool.tile([128, 1024], f32, name="ps", tag="ps")
                for nloc in range(2):
                    nb = nh * 2 + nloc
                    for kk in range(KP):
                        nc.tensor.matmul(
                            ps[:, nloc * 512:(nloc + 1) * 512],
                            qk[kk][:, :, m * 128:(m + 1) * 128],
                            pk[kk][:, :, nb * 512:(nb + 1) * 512],
                            start=(kk == 0),
                            stop=(kk == KP - 1),
                            perf_mode=DR,
                        )
                je = jpool.tile([128, 1024], bf16, name="je", tag="je")
                jc = jpool.tile([128, 1024], bf16, name="jc", tag="jc")
                nc.scalar.activation(
                    je[:], ps[:],
                    mybir.ActivationFunctionType.Exp,
                    bias=negc[:], scale=1.0,
                    accum_out=se_sb[:, u:u + 1],
                )
                if u in MASK_UNITS:
                    # units holding the self column: masked count
                    nc.vector.scalar_tensor_tensor(
                        out=jc[:], in0=ps[:],
                        scalar=tv[:, m:m + 1], in1=msk[:],
                        op0=mybir.AluOpType.is_gt,
                        op1=mybir.AluOpType.mult,
                        accum_out=cnt_sb[:, u:u + 1],
                    )
                else:
                    # NOTE: GPSIMD/Pool cannot read PSUM (BIR verifier), so
                    # every count runs on DVE.
                    nc.vector.tensor_scalar(
                        jc[:], ps[:], tv[:, m:m + 1], None,
                        op0=mybir.AluOpType.is_gt,
                        op1=mybir.AluOpType.add,
                        accum_out=cnt_sb[:, u:u + 1],
                    )

            nc.sync.dma_start(se_d[:], se_sb[:])
            nc.gpsimd.dma_start(cnt_d[:], cnt_sb[:])

    nc.compile()
    return nc


def _perm(c):
    """Rotation putting core c's own queries at m-tiles OWN_M, OWN_M+1."""
    return np.roll(np.arange(B), OWN_M * 128 - c * QSLAB)


def prepare(q, p):
    """Host-side shard prep. Returns (in_maps, t32, perms)."""
    import ml_dtypes
    fp8 = ml_dtypes.float8_e4m3
    q = np.ascontiguousarray(np.asarray(q, dtype=np.float32))
    p = np.ascontiguousarray(np.asarray(p, dtype=np.float32))

    # target scores t_i = q_i . p_{8i} (exact fp32; matches the reference's
    # value to ~1e-7 — only a compare threshold + host-tail term)
    t32 = np.einsum("ij,ij->i", q, p[::NP], dtype=np.float64).astype(np.float32)

    qT = np.ascontiguousarray(q.T)  # [D, B] fp32
    r = np.arange(128)
    # self columns: unit OWN_M has query pi=OWN_M*128+r vs in-unit col 8r
    # (nh 0); unit MT+OWN_M+1 has pi=(OWN_M+1)*128+r vs in-unit col 8r of
    # nh 1. Same mask for both, same for every core.
    msk = np.ones((128, 1024), dtype=np.float32)
    msk[r, 8 * r] = 0.0
    msk = msk.astype(ml_dtypes.bfloat16)

    in_maps = []
    perms = []
    for c in range(NCORES):
        perm = _perm(c)
        perms.append(perm)
        # [KP, 2, 128, B]: row kk*256 + i*128 + pp of qT (DoubleRow pairing)
        qTc = np.ascontiguousarray(qT[:, perm]).astype(fp8).reshape(
            KP, 2, 128, B)
        pTc = np.ascontiguousarray(
            p[c * PSLAB:(c + 1) * PSLAB].T).astype(fp8).reshape(
            KP, 2, 128, PSLAB)
        tvc = np.ascontiguousarray(t32[perm].reshape(MT, 128).T)
        in_maps.append({"qT": qTc, "pT": pTc, "tvec": tvc, "msk": msk})
    return in_maps, t32, perms


def finalize(results, t32, perms):
    """Combine per-core partials into the scalar loss (fp64 host tail)."""
    se_tot = np.zeros(B, dtype=np.float64)
    cnt_tot = np.zeros(B, dtype=np.float64)
    for c in range(NCORES):
        perm = perms[c]
        # col u = nh*MT + m; query pi = m*128 + r
        se = results[c]["se_out"].astype(np.float64)
        cnt = results[c]["cnt_out"].astype(np.float64)
        se_q = se.reshape(128, 2, MT).sum(axis=1).T.ravel()
        cnt_q = cnt.reshape(128, 2, MT).sum(axis=1).T.ravel()
        se_tot[perm] += se_q
        cnt_tot[perm] += cnt_q
    lse = C_SHIFT + np.log(se_tot)
    raw = lse - t32.astype(np.float64)
    w = 1.0 + ALPHA * np.exp(-((cnt_tot - OPTIMAL_RANK) ** 2)
                             / (2.0 * SIGMA ** 2))
    return np.float32(np.mean(raw * w))


def _get_nc():
    if "nc" not in _STATE:
        _STATE["nc"] = _build_nc()
    return _STATE["nc"]


def kernel(q_reps, p_reps, n_passages):
    assert int(np.asarray(n_passages)) == NP
    nc = _get_nc()
    in_maps, t32, perms = prepare(q_reps, p_reps)
    try:
        res = run_bass_kernel_spmd(nc, in_maps, core_ids=list(range(NCORES)))
    except Exception:
        # rare transient NRT_EXEC_UNIT_UNRECOVERABLE; reset the PJRT
        # client and retry once
        import time
        try:
            import jax
            jax.clear_caches()
            jax.extend.backend.clear_backends()
        except Exception:
            pass
        time.sleep(10)
        res = run_bass_kernel_spmd(nc, in_maps, core_ids=list(range(NCORES)))
    return finalize(res.results, t32, perms)


def run_profiled(q_reps, p_reps, n_passages, trace=True):
    """Same as kernel() but returns (loss, BassKernelResults) with NTFF
    profile (requires the antenv.axon_hooks shim; see _install_ntff_shim)."""
    nc = _get_nc()
    in_maps, t32, perms = prepare(q_reps, p_reps)
    res = run_bass_kernel_spmd(nc, in_maps, core_ids=list(range(NCORES)),
                               trace=trace)
    loss = finalize(res.results, t32, perms)
    return loss, res


def _install_ntff_shim():
    """Provide antenv.axon_hooks (absent in this image) so trace=True works."""
    import types
    import antenv
    if "antenv.axon_hooks" in sys.modules:
        return
    mod = types.ModuleType("antenv.axon_hooks")
    mod._hook = None
    mod.set_axon_ntff_profile_hook = lambda h: setattr(mod, "_hook", h)
    mod.get_axon_ntff_profile_hook = lambda: mod._hook
    sys.modules["antenv.axon_hooks"] = mod
    antenv.axon_hooks = mod
    try:
        from trn_agent_boot.trn_boot import _ntff_profile_via_ctypes
        hook = _ntff_profile_via_ctypes("/opt/axon/libaxon_pjrt.so")
        if hook is not None:
            mod._hook = hook
    except Exception:
        pass


# revision 14
# speedup vs baseline: 1.6829x; 1.0735x over previous
"""Trainium2 Bass kernel for nn_DenseModel_51926154609008 (weighted-rank
contrastive CE loss).

Math (reference semantics, no sort needed):
  scores = q @ p.T                       [B=2048, P=16384]
  t_i    = scores[i, 8*i]                (positive/target score)
  rank_i = #{j : scores[i, j] > t_i}     (argsort position == exceed count,
                                          ties are measure-zero for randn data)
  lse_i  = logsumexp(scores[i, :])
  loss   = mean((lse_i - t_i) * (1 + 2.6*exp(-(rank_i-1)^2 / (2*1.8^2))))

Sharding: passage-parallel (P split across 8 cores, q replicated). Each core
computes a [2048, 2048] score slab in 32 units of [128 queries x 1024
passages] (one 2-bank PSUM buffer each) and reduces every unit to per-query
partials:
  sumexp_c[i] = sum_j exp(s_ij - C)      (fixed shift C so partials add
                                          across cores without a max-merge)
  cnt_c[i]    = #{j in slab : s_ij > t_i}
The host combines partials and evaluates the tiny [2048] tail in fp64.

Matmuls run in fp8-e4m3 with MatmulPerfMode.DoubleRow: one MM contracts TWO
128-deep k-chunks (lhsT [128,2,128], rhs [128,2,1024] -> out [128,1024] =
2 PSUM banks), 2x the bf16 PE throughput. The wide (1024-col) output makes
the 427ns stream hide the 156ns LDWEIGHTS that was gating back-to-back
512-col MMs (measured 262ns cadence for a 216ns stream). End-to-end fp8
quantization error on this exact input is rel 3.4e-4 on the loss (verified
numerically) vs the 2e-2 gate.

Per unit, stats run at full [128,1024] granularity (2 PSUM banks):
  - ACT: exp(s - C) with sum-accumulator -> se col (one op, ~1.3us). ACT is
    the ONLY PSUM drain, so PSUM slot release = ACT completion.
  - DVE: count runs on ACT's bf16 exp OUTPUT in SBUF, not on PSUM:
    #(s > t) == #(exp(s-C) > exp(t-C)) by monotonicity. All-SBUF 2-byte
    operands enable the DVE 4X perf mode (~0.4us vs ~1.3us from PSUM).
    exp underflow (s-C < -87 flushes to 0) only corrupts counts for
    queries whose t is itself far below the top — their true rank is
    O(hundreds+) either way and the Gaussian rank weight is exactly 1,
    so the loss is unaffected (verified numerically).
    (Pool/GPSIMD cannot read PSUM, so it can't share drain work.)

The self-comparison (j == 8i) must contribute exactly 0 to rank_i. Query i's
target column lives only in core (i//256)'s slab. Each core rotates its query
order (data-level permutation, program stays SPMD-uniform) so its own queries
land on m-tiles OWN_M, OWN_M+1; the two units containing self-columns use a
masked count (indicator * mask, one fused DVE op).

t itself is computed on the host (trivial 2048x768 row-dot, exact fp32).

HW notes baked in from trace/bisect evidence (previous sessions):
  - DVE ops fault when an access pattern spans >2 PSUM banks; 2 banks is
    fine -> one [128,1024] count op per unit.
  - ACT reads spanning 4 banks are fine; exp uses the per-instruction
    accumulator (sum along free dim) so no junk reduction is needed.
  - Mixing ACT functions forces ~1.3us ACT_TABLE_LOADs; Exp only here.
  - DMA issue is ~0.6us per dma_start on the issuing sequencer, so the
    critical first operands go first, split across four sequencers.
"""

import sys

import numpy as np

sys.path.insert(0, "/opt/trn_rl_repo")

import concourse.bacc as bacc  # noqa: E402
import concourse.bass as bass  # noqa: E402
import concourse.mybir as mybir  # noqa: E402
import concourse.tile as tile  # noqa: E402
from concourse.bass_utils import run_bass_kernel_spmd  # noqa: E402

# Problem shape (hardcoded per the task contract).
B = 2048
D = 768
NP = 8
P = B * NP  # 16384
NCORES = 8
PSLAB = P // NCORES  # 2048 passage columns per core
KP = D // 256  # 3 DoubleRow k-pairs (each = two 128-deep chunks)
MT = B // 128  # 16 query m-tiles
NU = 2 * MT  # 32 units of [128, 1024]
QSLAB = B // NCORES  # 256 queries owned per core
OWN_M = 8  # own queries sit at m-tiles 8,9
MASK_UNITS = (OWN_M, MT + OWN_M + 1)  # units holding self-columns

C_SHIFT = 128.0  # fixed exp shift: exp(s - C) never overflows for this data

ALPHA = 2.6
OPTIMAL_RANK = 1.0
SIGMA = 1.8

_STATE: dict = {}


def _set_no_ldweights(mm):
    """Mark an InstMatmult as non-self-loading: it reuses the PE weights
    loaded by the immediately preceding matmul (identical lhsT AP). PE
    instructions execute in program order, so the pairing is stable."""
    mm.ins.ldweights = False


def _build_nc():
    nc = bacc.Bacc("TRN2", target_bir_lowering=False, debug=False,
                   num_devices=NCORES)

    fp8 = mybir.dt.float8e4
    f32 = mybir.dt.float32
    bf16 = mybir.dt.bfloat16
    DR = mybir.MatmulPerfMode.DoubleRow

    qT_d = nc.dram_tensor("qT", [KP, 2, 128, B], fp8, kind="ExternalInput").ap()
    pT_d = nc.dram_tensor("pT", [KP, 2, 128, PSLAB], fp8,
                          kind="ExternalInput").ap()
    tv_d = nc.dram_tensor("expt", [128, MT], f32, kind="ExternalInput").ap()
    msk_d = nc.dram_tensor("msk", [128, 1024], bf16, kind="ExternalInput").ap()
    se_d = nc.dram_tensor("se_out", [128, NU], f32, kind="ExternalOutput").ap()
    cnt_d = nc.dram_tensor("cnt_out", [128, NU], f32,
                           kind="ExternalOutput").ap()

    with tile.TileContext(nc) as tc:
        with (
            tc.tile_pool(name="weights", bufs=1) as wpool,
            tc.tile_pool(name="stats", bufs=1) as spool,
            tc.tile_pool(name="junk", bufs=3) as jpool,
            tc.tile_pool(name="psum", bufs=4,
                         space=bass.MemorySpace.PSUM) as ppool,
        ):
            qk = [wpool.tile([128, 2, B], fp8, name=f"qk{k}", tag=f"qk{k}")
                  for k in range(KP)]
            pk = [wpool.tile([128, 2, PSLAB], fp8, name=f"pk{k}", tag=f"pk{k}")
                  for k in range(KP)]
            tv = spool.tile([128, MT], f32, name="tv", tag="tv")
            msk = spool.tile([128, 1024], bf16, name="msk", tag="msk")
            negc = spool.tile([128, 1], f32, name="negc", tag="negc")
            se_sb = spool.tile([128, NU], f32, name="se_sb", tag="se_sb")
            cnt_sb = spool.tile([128, NU], f32, name="cnt_sb", tag="cnt_sb")

            def ldq(eng, kk, i, c0, c1):
                eng.dma_start(qk[kk][:, i, c0:c1], qT_d[kk, i, :, c0:c1])

            def ldp(eng, kk, i, c0, c1):
                eng.dma_start(pk[kk][:, i, c0:c1], pT_d[kk, i, :, c0:c1])

            # negc must precede the first ACT op; DVE engine op, trivial.
            nc.vector.memset(negc[:], -C_SHIFT)

            # DMA issue is ~0.6us per dma_start on its sequencer, and only
            # sync/scalar/gpsimd sequencers can issue. Order per sequencer
            # by first-use time: unit u=(nh,m) consumes pk cols
            # [nh*1024, (nh+1)*1024) (both i) and qk cols [m*128,(m+1)*128);
            # units run nh-major.
            # sync carries pk i=0, scalar carries pk i=1, both in first-use
            # order (kk-major MMs touch bank 0 and bank 1 of each kk
            # back-to-back, so banks interleave per kk). tv early (first
            # DVE count at ~PE start + 1.5us), msk later (unit 8).
            ldp(nc.sync, 0, 0, 0, 512)
            ldp(nc.sync, 0, 0, 512, 1024)
            ldp(nc.sync, 1, 0, 0, 512)
            ldp(nc.sync, 1, 0, 512, 1024)
            nc.sync.dma_start(tv[:], tv_d[:])
            ldp(nc.sync, 2, 0, 0, 512)
            ldp(nc.sync, 2, 0, 512, 1024)
            ldp(nc.sync, 0, 0, 1024, 2048)
            ldp(nc.sync, 1, 0, 1024, 2048)
            ldp(nc.sync, 2, 0, 1024, 2048)
            # scalar: pk i=1 + msk, then the sequencer is clear for ACT
            ldp(nc.scalar, 0, 1, 0, 512)
            ldp(nc.scalar, 0, 1, 512, 1024)
            ldp(nc.scalar, 1, 1, 0, 512)
            ldp(nc.scalar, 1, 1, 512, 1024)
            ldp(nc.scalar, 2, 1, 0, 512)
            ldp(nc.scalar, 2, 1, 512, 1024)
            nc.scalar.dma_start(msk[:], msk_d[:])
            # gpsimd: all qk chunks (m0-1, then m2-7, then m8-15), late pk
            for kk in range(KP):
                ldq(nc.gpsimd, kk, 0, 0, 256)
                ldq(nc.gpsimd, kk, 1, 0, 256)
            for kk in range(KP):
                ldq(nc.gpsimd, kk, 0, 256, 1024)
                ldq(nc.gpsimd, kk, 1, 256, 1024)
            for kk in range(KP):
                ldq(nc.gpsimd, kk, 0, 1024, 2048)
                ldq(nc.gpsimd, kk, 1, 1024, 2048)
            ldp(nc.gpsimd, 0, 1, 1024, 2048)
            ldp(nc.gpsimd, 1, 1, 1024, 2048)
            ldp(nc.gpsimd, 2, 1, 1024, 2048)

            for u in range(NU):
                nh, m = u // MT, u % MT
                ps = ppool.tile([128, 1024], f32, name="ps", tag="ps")
                # kk-major: consecutive bank pairs share the same lhsT
                # weights (qk[kk][m]); the second MM of each pair skips its
                # LDWEIGHTS (ldweights=False) — the 156ns DR weight load
                # otherwise gates the 213ns MM stream to a 262ns cadence.
                for kk in range(KP):
                    for nloc in range(2):
                        nb = nh * 2 + nloc
                        mm = nc.tensor.matmul(
                            ps[:, nloc * 512:(nloc + 1) * 512],
                            qk[kk][:, :, m * 128:(m + 1) * 128],
                            pk[kk][:, :, nb * 512:(nb + 1) * 512],
                            start=(kk == 0),
                            stop=(kk == KP - 1),
                            perf_mode=DR,
                        )
                        if nloc == 1:
                            _set_no_ldweights(mm)
                je = jpool.tile([128, 1024], bf16, name="je", tag="je")
                jc = jpool.tile([128, 1024], bf16, name="jc", tag="jc")
                nc.scalar.activation(
                    je[:], ps[:],
                    mybir.ActivationFunctionType.Exp,
                    bias=negc[:], scale=1.0,
                    accum_out=se_sb[:, u:u + 1],
                )
                # count on je (bf16, SBUF) so DVE gets the 4X mode and PSUM
                # drains via ACT alone
                if u in MASK_UNITS:
                    # units holding the self column: masked count (no DVE
                    # perf mode for scalar_tensor_tensor; 2 units only)
                    nc.vector.scalar_tensor_tensor(
                        out=jc[:], in0=je[:],
                        scalar=tv[:, m:m + 1], in1=msk[:],
                        op0=mybir.AluOpType.is_gt,
                        op1=mybir.AluOpType.mult,
                        accum_out=cnt_sb[:, u:u + 1],
                    )
                else:
                    nc.vector.tensor_scalar(
                        jc[:], je[:], tv[:, m:m + 1], None,
                        op0=mybir.AluOpType.is_gt,
                        op1=mybir.AluOpType.add,
                        accum_out=cnt_sb[:, u:u + 1],
                    )

            nc.sync.dma_start(se_d[:], se_sb[:])
            nc.gpsimd.dma_start(cnt_d[:], cnt_sb[:])

    nc.compile()
    return nc


def _perm(c):
    """Rotation putting core c's own queries at m-tiles OWN_M, OWN_M+1."""
    return np.roll(np.arange(B), OWN_M * 128 - c * QSLAB)


def prepare(q, p):
    """Host-side shard prep. Returns (in_maps, t32, perms)."""
    import ml_dtypes
    fp8 = ml_dtypes.float8_e4m3
    q = np.ascontiguousarray(np.asarray(q, dtype=np.float32))
    p = np.ascontiguousarray(np.asarray(p, dtype=np.float32))

    # target scores t_i = q_i . p_{8i} (exact fp32; matches the reference's
    # value to ~1e-7 — only a compare threshold + host-tail term)
    t64 = np.einsum("ij,ij->i", q, p[::NP], dtype=np.float64)
    t32 = t64.astype(np.float32)
    # DVE count threshold: exp(t - C), compared against ACT's bf16 exp output
    expt = np.exp(t64 - C_SHIFT).astype(np.float32)

    qT = np.ascontiguousarray(q.T)  # [D, B] fp32
    r = np.arange(128)
    # self columns: unit OWN_M has query pi=OWN_M*128+r vs in-unit col 8r
    # (nh 0); unit MT+OWN_M+1 has pi=(OWN_M+1)*128+r vs in-unit col 8r of
    # nh 1. Same mask for both, same for every core.
    msk = np.ones((128, 1024), dtype=np.float32)
    msk[r, 8 * r] = 0.0
    msk = msk.astype(ml_dtypes.bfloat16)

    in_maps = []
    perms = []
    for c in range(NCORES):
        perm = _perm(c)
        perms.append(perm)
        # [KP, 2, 128, B]: row kk*256 + i*128 + pp of qT (DoubleRow pairing)
        qTc = np.ascontiguousarray(qT[:, perm]).astype(fp8).reshape(
            KP, 2, 128, B)
        pTc = np.ascontiguousarray(
            p[c * PSLAB:(c + 1) * PSLAB].T).astype(fp8).reshape(
            KP, 2, 128, PSLAB)
        tvc = np.ascontiguousarray(expt[perm].reshape(MT, 128).T)
        in_maps.append({"qT": qTc, "pT": pTc, "expt": tvc, "msk": msk})
    return in_maps, t32, perms


def finalize(results, t32, perms):
    """Combine per-core partials into the scalar loss (fp64 host tail)."""
    se_tot = np.zeros(B, dtype=np.float64)
    cnt_tot = np.zeros(B, dtype=np.float64)
    for c in range(NCORES):
        perm = perms[c]
        # col u = nh*MT + m; query pi = m*128 + r
        se = results[c]["se_out"].astype(np.float64)
        cnt = results[c]["cnt_out"].astype(np.float64)
        se_q = se.reshape(128, 2, MT).sum(axis=1).T.ravel()
        cnt_q = cnt.reshape(128, 2, MT).sum(axis=1).T.ravel()
        se_tot[perm] += se_q
        cnt_tot[perm] += cnt_q
    lse = C_SHIFT + np.log(se_tot)
    raw = lse - t32.astype(np.float64)
    w = 1.0 + ALPHA * np.exp(-((cnt_tot - OPTIMAL_RANK) ** 2)
                             / (2.0 * SIGMA ** 2))
    return np.float32(np.mean(raw * w))


def _get_nc():
    if "nc" not in _STATE:
        _STATE["nc"] = _build_nc()
    return _STATE["nc"]


def kernel(q_reps, p_reps, n_passages):
    assert int(np.asarray(n_passages)) == NP
    nc = _get_nc()
    in_maps, t32, perms = prepare(q_reps, p_reps)
    try:
        res = run_bass_kernel_spmd(nc, in_maps, core_ids=list(range(NCORES)))
    except Exception:
        # rare transient NRT_EXEC_UNIT_UNRECOVERABLE; reset the PJRT
        # client and retry once
        import time
        try:
            import jax
            jax.clear_caches()
            jax.extend.backend.clear_backends()
        except Exception:
            pass
        time.sleep(10)
        res = run_bass_kernel_spmd(nc, in_maps, core_ids=list(range(NCORES)))
    return finalize(res.results, t32, perms)


def run_profiled(q_reps, p_reps, n_passages, trace=True):
    """Same as kernel() but returns (loss, BassKernelResults) with NTFF
    profile (requires the antenv.axon_hooks shim; see _install_ntff_shim)."""
    nc = _get_nc()
    in_maps, t32, perms = prepare(q_reps, p_reps)
    res = run_bass_kernel_spmd(nc, in_maps, core_ids=list(range(NCORES)),
                               trace=trace)
    loss = finalize(res.results, t32, perms)
    return loss, res


def _install_ntff_shim():
    """Provide antenv.axon_hooks (absent in this image) so trace=True works."""
    import types
    import antenv
    if "antenv.axon_hooks" in sys.modules:
        return
    mod = types.ModuleType("antenv.axon_hooks")
    mod._hook = None
    mod.set_axon_ntff_profile_hook = lambda h: setattr(mod, "_hook", h)
    mod.get_axon_ntff_profile_hook = lambda: mod._hook
    sys.modules["antenv.axon_hooks"] = mod
    antenv.axon_hooks = mod
    try:
        from trn_agent_boot.trn_boot import _ntff_profile_via_ctypes
        hook = _ntff_profile_via_ctypes("/opt/axon/libaxon_pjrt.so")
        if hook is not None:
            mod._hook = hook
    except Exception:
        pass
